# revision 9
# baseline (speedup 1.0000x reference)
"""GAU (Gated Attention Unit) Trainium2 kernel, 8-core SPMD.

Sharding: core c -> (batch b = c//2, hidden-slice h = c%2).
Each core computes, for its batch, the full causal attention with the
hidden dim (2048) split in half (1024 per core); W_out row-split gives
partial outputs that the host sums (+ residual is added on-device by
both halves? no: device adds x only via ph3 epilogue on BOTH partials
would double it -- device adds x*0.5? No: we add x on device in NEITHER
partial; host adds x and b_out at gather time... see _gather()).

Per-core pipeline (all matmuls bf16 with fp32 PSUM accumulation):
  ph1: LN(x) row-tiles -> PE-transpose -> normedT; projections:
       v (row-major, SBUF-resident), gateT (-> DRAM scratch),
       qT/kT (qk_dim=128 on partitions; q pre-scaled by 1/(i+1)).
  ph2: per 512-col i-chunk: simT[j,i] = kT.T@qT -> relu -> square
       (-> causal mask on diagonal tiles) cached as attnT; then
       oT[vd,i] accumulated over j in two 4-bank PSUM groups;
       gate-multiplied -> ogT (SBUF-resident).
  ph3: out[rows,dim] = ogT.T @ W_out_slice, +x residual, -> DRAM.
"""

import time
import numpy as np
import ml_dtypes

import concourse.bacc as bacc
import concourse.tile as tile
import concourse.bass as bass
from concourse import mybir
from concourse import bass_utils

F32 = mybir.dt.float32
BF16 = mybir.dt.bfloat16
BF16_NP = ml_dtypes.bfloat16
AF = mybir.ActivationFunctionType
ALU = mybir.AluOpType

B, N, DIM, QK, HID = 4, 4096, 1024, 128, 2048
NCORES = 8
RT = N // 128      # 32 row tiles
NCH = N // 512     # 8 row chunks
DT = DIM // 128    # 8 dim tiles
HSL = HID // 2     # 1024 per-core hidden slice
VD = HSL // 128    # 8 vd tiles

LAST_EXEC_S = None
_PROG = None


def _build_program():
    nc = bacc.Bacc("TRN2", target_bir_lowering=False, debug=False,
                   num_devices=NCORES)

    x_d = nc.dram_tensor("x", [N, DIM], F32, kind="ExternalInput")
    wh_d = nc.dram_tensor("wh", [DIM, 2 * HSL], BF16, kind="ExternalInput")
    wqk_d = nc.dram_tensor("wqk", [DIM, QK], BF16, kind="ExternalInput")
    wout_d = nc.dram_tensor("wout", [HSL, DIM], BF16, kind="ExternalInput")
    g0_d = nc.dram_tensor("g0", [QK], F32, kind="ExternalInput")
    g1_d = nc.dram_tensor("g1", [QK], F32, kind="ExternalInput")
    bt0_d = nc.dram_tensor("bt0", [QK], F32, kind="ExternalInput")
    bt1_d = nc.dram_tensor("bt1", [QK], F32, kind="ExternalInput")
    rsc_d = nc.dram_tensor("rsc", [N], BF16, kind="ExternalInput")
    mask_d = nc.dram_tensor("mask", [128, 896], BF16, kind="ExternalInput")
    ident_d = nc.dram_tensor("ident", [128, 128], BF16, kind="ExternalInput")
    out_d = nc.dram_tensor("out", [N, DIM], F32, kind="ExternalOutput")
    # DRAM scratch for gate in transposed layout [vd_tile, 128, rows]
    gT_d = nc.dram_tensor("gT_scratch", [VD, 128, N], BF16)

    x_rows = x_d.ap().rearrange("(rt p) d -> rt p d", p=128)
    out_rows = out_d.ap().rearrange("(rt p) d -> rt p d", p=128)
    wh_r = wh_d.ap().rearrange("(dt p) c -> p dt c", p=128)
    wqk_r = wqk_d.ap().rearrange("(dt p) q -> p dt q", p=128)
    wout_r = wout_d.ap().rearrange("(kt p) d -> p kt d", p=128)

    def bcast_ap(t, n):
        a = t.ap()
        return bass.AP(tensor=a.tensor, offset=a.offset, ap=[[0, 128]] + a.ap)

    with tile.TileContext(nc) as tc:
        with tc.tile_pool(name="consts", bufs=1) as consts, \
             tc.tile_pool(name="vres", bufs=1) as vres, \
             tc.tile_pool(name="qkres", bufs=1) as qkres:
            ident_sb = consts.tile([128, 128], BF16, tag="ident")
            nc.sync.dma_start(out=ident_sb, in_=ident_d.ap())
            mask_sb = consts.tile([128, 896], BF16, tag="mask")
            nc.sync.dma_start(out=mask_sb, in_=mask_d.ap())
            rsc_sb = consts.tile([128, N], BF16, tag="rsc")
            nc.gpsimd.dma_start(out=rsc_sb, in_=bcast_ap(rsc_d, N))
            g0_sb = consts.tile([128, 1], F32, tag="g0")
            nc.sync.dma_start(out=g0_sb, in_=g0_d.ap().rearrange("(p o) -> p o", o=1))
            g1_sb = consts.tile([128, 1], F32, tag="g1")
            nc.sync.dma_start(out=g1_sb, in_=g1_d.ap().rearrange("(p o) -> p o", o=1))
            bt0_sb = consts.tile([128, 1], F32, tag="bt0")
            nc.sync.dma_start(out=bt0_sb, in_=bt0_d.ap().rearrange("(p o) -> p o", o=1))
            bt1_sb = consts.tile([128, 1], F32, tag="bt1")
            nc.sync.dma_start(out=bt1_sb, in_=bt1_d.ap().rearrange("(p o) -> p o", o=1))
            eps_sb = consts.tile([128, 1], F32, tag="eps")
            nc.vector.memset(eps_sb, 1e-5)

            v_sb = vres.tile([128, RT, HSL], BF16, tag="v")
            qT_sb = qkres.tile([128, N], BF16, tag="qT")
            kT_sb = qkres.tile([128, N], BF16, tag="kT")

            # ---------------- phase 1: LN + projections ----------------
            with tc.tile_pool(name="ph1w", bufs=1) as ph1w, \
                 tc.tile_pool(name="ph1s", bufs=3) as ph1s, \
                 tc.tile_pool(name="ph1nt", bufs=2) as ph1nt, \
                 tc.tile_pool(name="ps_t", bufs=2, space="PSUM") as ps_t, \
                 tc.tile_pool(name="ps_v", bufs=2, space="PSUM") as ps_v, \
                 tc.tile_pool(name="ps_g", bufs=2, space="PSUM") as ps_g, \
                 tc.tile_pool(name="ps_qk", bufs=2, space="PSUM") as ps_qk:
                wh_sb = ph1w.tile([128, DT, 2 * HSL], BF16, tag="wh")
                nc.sync.dma_start(out=wh_sb, in_=wh_r)
                wqk_sb = ph1w.tile([128, DT, QK], BF16, tag="wqk")
                nc.sync.dma_start(out=wqk_sb, in_=wqk_r)

                for ch in range(NCH):
                    r0 = ch * 512
                    nT = ph1nt.tile([128, DT, 512], BF16, tag="nT")
                    for rt4 in range(4):
                        rt = ch * 4 + rt4
                        xt = ph1s.tile([128, DIM], F32, tag="xt")
                        nc.sync.dma_start(out=xt, in_=x_rows[rt])
                        st = ph1s.tile([128, 2, 6], F32, tag="st")
                        nc.vector.bn_stats(out=st[:, 0, :], in_=xt[:, 0:512])
                        nc.vector.bn_stats(out=st[:, 1, :], in_=xt[:, 512:1024])
                        mv = ph1s.tile([128, 2], F32, tag="mv")
                        nc.vector.bn_aggr(out=mv, in_=st)
                        rstd = ph1s.tile([128, 1], F32, tag="rstd")
                        nc.scalar.activation(out=rstd, in_=mv[:, 1:2],
                                             func=AF.Sqrt, bias=eps_sb, scale=1.0)
                        nc.vector.reciprocal(out=rstd, in_=rstd)
                        nm = ph1s.tile([128, DIM], BF16, tag="nm")
                        nc.vector.tensor_scalar(nm, xt, mv[:, 0:1], rstd,
                                                ALU.subtract, ALU.mult)
                        for dt in range(DT):
                            pt = ps_t.tile([128, 128], BF16, tag="pt")
                            nc.tensor.transpose(pt, nm[:, dt * 128:(dt + 1) * 128],
                                                ident_sb)
                            nc.any.tensor_copy(
                                out=nT[:, dt, rt4 * 128:(rt4 + 1) * 128], in_=pt)

                    # qk projection -> qT/kT slices
                    qkps = ps_qk.tile([128, 512], F32, tag="qkps")
                    for dt in range(DT):
                        nc.tensor.matmul(qkps, lhsT=wqk_sb[:, dt, :],
                                         rhs=nT[:, dt, :],
                                         start=(dt == 0), stop=(dt == DT - 1))
                    qsil = ph1s.tile([128, 512], BF16, tag="qsil")
                    nc.scalar.activation(out=qsil, in_=qkps, func=AF.Silu)
                    nc.vector.tensor_scalar(kT_sb[:, r0:r0 + 512], qsil,
                                            g1_sb, bt1_sb, ALU.mult, ALU.add)
                    qtmp = ph1s.tile([128, 512], BF16, tag="qtmp")
                    nc.vector.tensor_scalar(qtmp, qsil, g0_sb, bt0_sb,
                                            ALU.mult, ALU.add)
                    nc.vector.tensor_mul(qT_sb[:, r0:r0 + 512], qtmp,
                                         rsc_sb[:, r0:r0 + 512])

                    # v projection, row-major [rows, vcol]
                    for rt4 in range(4):
                        rt = ch * 4 + rt4
                        for vc in range(2):
                            vps = ps_v.tile([128, 512], F32, tag="vps")
                            for dt in range(DT):
                                nc.tensor.matmul(
                                    vps,
                                    lhsT=nT[:, dt, rt4 * 128:(rt4 + 1) * 128],
                                    rhs=wh_sb[:, dt, vc * 512:(vc + 1) * 512],
                                    start=(dt == 0), stop=(dt == DT - 1))
                            nc.scalar.activation(
                                out=v_sb[:, rt, vc * 512:(vc + 1) * 512],
                                in_=vps, func=AF.Silu)

                    # gate projection, transposed [gcol, rows] -> DRAM
                    for gc in range(VD):
                        gps = ps_g.tile([128, 512], F32, tag="gps")
                        for dt in range(DT):
                            nc.tensor.matmul(
                                gps,
                                lhsT=wh_sb[:, dt,
                                           HSL + gc * 128:HSL + (gc + 1) * 128],
                                rhs=nT[:, dt, :],
                                start=(dt == 0), stop=(dt == DT - 1))
                        gsb = ph1s.tile([128, 512], BF16, tag="gsb")
                        nc.scalar.activation(out=gsb, in_=gps, func=AF.Silu)
                        nc.sync.dma_start(out=gT_d.ap()[gc, :, r0:r0 + 512],
                                          in_=gsb)

            # ---------------- phase 2: attention ----------------
            with tc.tile_pool(name="ogres", bufs=1) as ogres:
                ogT_sb = ogres.tile([128, VD, N], BF16, tag="ogT")
                with tc.tile_pool(name="ph2at", bufs=1) as ph2at, \
                     tc.tile_pool(name="ph2s", bufs=4) as ph2s, \
                     tc.tile_pool(name="ps_sim", bufs=2, space="PSUM") as ps_sim, \
                     tc.tile_pool(name="ps_ot", bufs=1, space="PSUM") as ps_ot:
                    _phase2(nc, tc, ps_sim, ps_ot, ph2at, ph2s,
                            qT_sb, kT_sb, v_sb, ogT_sb, mask_sb, gT_d)

                # ---------------- phase 3: out projection ----------------
                with tc.tile_pool(name="ph3w", bufs=1) as ph3w, \
                     tc.tile_pool(name="ph3s", bufs=3) as ph3s, \
                     tc.tile_pool(name="ps_out", bufs=4, space="PSUM") as ps_out:
                    wout_sb = ph3w.tile([128, DT, DIM], BF16, tag="wout")
                    nc.sync.dma_start(out=wout_sb, in_=wout_r)
                    for rt in range(RT):
                        xt3 = ph3s.tile([128, DIM], F32, tag="xt3")
                        nc.sync.dma_start(out=xt3, in_=x_rows[rt])
                        ot = ph3s.tile([128, DIM], F32, tag="ot")
                        for dh in range(2):
                            ops = ps_out.tile([128, 512], F32, tag="ops")
                            for kt in range(DT):
                                nc.tensor.matmul(
                                    ops,
                                    lhsT=ogT_sb[:, kt, rt * 128:(rt + 1) * 128],
                                    rhs=wout_sb[:, kt, dh * 512:(dh + 1) * 512],
                                    start=(kt == 0), stop=(kt == DT - 1))
                            # each of the 2 cores of a batch pair adds x/2 so
                            # the host-side partial sum carries exactly one x
                            nc.vector.scalar_tensor_tensor(
                                ot[:, dh * 512:(dh + 1) * 512],
                                xt3[:, dh * 512:(dh + 1) * 512], 0.5, ops,
                                ALU.mult, ALU.add)
                        nc.sync.dma_start(out=out_rows[rt], in_=ot)

    nc.compile()
    return nc


def _phase2(nc, tc, ps_sim, ps_ot, ph2at, ph2s, qT_sb, kT_sb, v_sb, ogT_sb,
            mask_sb, gT_d):
                for ic in range(NCH):
                    c0 = ic * 512
                    jtmax = 4 * ic + 4
                    atile = ph2at.tile([128, RT, 512], BF16, tag="atile")
                    for jt in range(jtmax):
                        sps = ps_sim.tile([128, 512], F32, tag="sps")
                        nc.tensor.matmul(sps,
                                         lhsT=kT_sb[:, jt * 128:(jt + 1) * 128],
                                         rhs=qT_sb[:, c0:c0 + 512],
                                         start=True, stop=True)
                        rl = ph2s.tile([128, 512], BF16, tag="rl")
                        nc.scalar.activation(out=rl, in_=sps, func=AF.Relu)
                        off = jt * 128 - c0
                        if off >= 0:
                            nc.vector.tensor_mul(atile[:, jt, :], rl, rl)
                            nc.vector.tensor_mul(
                                atile[:, jt, :], atile[:, jt, :],
                                mask_sb[:, 384 - off:896 - off])
                        else:
                            nc.vector.tensor_mul(atile[:, jt, :], rl, rl)
                    for half in range(2):
                        pst = [ps_ot.tile([128, 512], F32, tag=f"pst{q}",
                                          name=f"pst{q}_{ic}_{half}")
                               for q in range(4)]
                        for jt in range(jtmax):
                            for q in range(4):
                                vd = half * 4 + q
                                nc.tensor.matmul(
                                    pst[q],
                                    lhsT=v_sb[:, jt, vd * 128:(vd + 1) * 128],
                                    rhs=atile[:, jt, :],
                                    start=(jt == 0), stop=(jt == jtmax - 1))
                        for q in range(4):
                            vd = half * 4 + q
                            gsl = ph2s.tile([128, 512], BF16, tag="gsl")
                            nc.sync.dma_start(out=gsl,
                                              in_=gT_d.ap()[vd, :, c0:c0 + 512])
                            nc.vector.tensor_mul(ogT_sb[:, vd, c0:c0 + 512],
                                                 pst[q], gsl)


def _get_program():
    global _PROG
    if _PROG is None:
        _PROG = _build_program()
    return _PROG


def kernel(x, ln_g, ln_b, W_hidden, b_hidden, W_qk, b_qk, os_gamma, os_beta,
           W_out, b_out):
    global LAST_EXEC_S
    x = np.asarray(x, np.float32)
    ln_g = np.asarray(ln_g, np.float32)
    ln_b = np.asarray(ln_b, np.float32)
    W_hidden = np.asarray(W_hidden, np.float32)
    W_qk = np.asarray(W_qk, np.float32)
    os_gamma = np.asarray(os_gamma, np.float32)
    os_beta = np.asarray(os_beta, np.float32)
    W_out = np.asarray(W_out, np.float32)

    assert not np.any(ln_b), "nonzero ln_b not supported by folded weights"
    assert not np.any(np.asarray(b_hidden)), "nonzero b_hidden unsupported"
    assert not np.any(np.asarray(b_qk)), "nonzero b_qk unsupported"

    # fold LN gain into the projection weights
    Wh = (W_hidden * ln_g[:, None])
    Wq = (W_qk * ln_g[:, None]).astype(BF16_NP)

    rsc = (1.0 / (np.arange(N, dtype=np.float64) + 1.0)).astype(BF16_NP)
    jj = np.arange(128)[:, None]
    cc = np.arange(896)[None, :]
    mask = (jj <= cc - 384).astype(BF16_NP)
    ident = np.eye(128, dtype=BF16_NP)

    nc = _get_program()

    in_maps = []
    for c in range(NCORES):
        b, h = divmod(c, 2)
        wh_c = np.ascontiguousarray(
            np.concatenate([Wh[:, h * HSL:(h + 1) * HSL],
                            Wh[:, HID + h * HSL:HID + (h + 1) * HSL]],
                           axis=1)).astype(BF16_NP)
        wout_c = np.ascontiguousarray(W_out[h * HSL:(h + 1) * HSL, :]).astype(BF16_NP)
        in_maps.append({
            "x": np.ascontiguousarray(x[b]),
            "wh": wh_c,
            "wqk": Wq,
            "wout": wout_c,
            "g0": np.ascontiguousarray(os_gamma[0]),
            "g1": np.ascontiguousarray(os_gamma[1]),
            "bt0": np.ascontiguousarray(os_beta[0]),
            "bt1": np.ascontiguousarray(os_beta[1]),
            "rsc": rsc,
            "mask": mask,
            "ident": ident,
        })

    t0 = time.time()
    res = bass_utils.run_bass_kernel_spmd(nc, in_maps,
                                          core_ids=list(range(NCORES)))
    LAST_EXEC_S = time.time() - t0

    b_out = np.asarray(b_out, np.float32)
    out = np.empty((B, N, DIM), np.float32)
    for b in range(B):
        out[b] = res.results[2 * b]["out"] + res.results[2 * b + 1]["out"] + b_out
    return out
